# revision 26
# baseline (speedup 1.0000x reference)
"""Trainium2 kernel for MinibatchDiscrimination.

reference:
    M = einsum('ni,ibk->nbk', x, T)            # (256, 256, 16)
    l1[n,m,b] = sum_k |M[n,b,k] - M[m,b,k]|
    out[m,b]  = sum_n exp(-l1[n,m,b]) - 1      # (256, 256)
    return concat([x, out], axis=1)            # (256, 1280)

Sharding: tensor-parallel over the B_extra=256 feature dim -> 32 features
per core, no collectives. Each core computes out[:, shard] as [32, 256]
(batch on partitions), host transposes and concatenates with x.

Per-core dataflow (upper triangle, batched groups):
  |a-b| = 2*max(a,b) - a - b, so
  l1[n,m,b] = 2*sum_k max(M[n,bk], M[m,bk]) - SS[b,n] - SS[b,m]
  with SS[b,m] = sum_k M[m,b,k] (selector matmul on M).

  Groups of P consecutive n's (P in {8,16,32}, growing as w = 256-n0
  shrinks). Per group:
    DVE: 4 scalar_tensor_tensor ops (one per 128-partition (b,k) chunk):
         r[:, c, j, m] = max(2*M[:, m], 2*M[:, n0+j]) via broadcast APs,
         fp16.
    PE (per PSUM bank of r_b = floor(512/w) rows): -SS[m] and -SS[n]
         corrections plus a poison matmul (+1e4 at m <= n: kills
         sub-diagonal and self terms after exp), then 4 selector
         matmuls contracting the (b,k) partitions of r. All correction/
         accumulation matmuls use 128-row zero-padded stationaries and
         moving tiles: 32-row contractions stream at half rate.
    ACT: one exp per bank: e = exp(-psum), fp16, poisoned entries -> 0.
    DVE: one segmented tensor_reduce per group: row sums
         accn[b, n0+j] = sum_m e[b, j, m].
    PE:  per bank, a column-accumulation matmul with a j-broadcast
         (0-stride) PSUM out AP: acc[b, m] += sum_j e[b, j, m].
  out_dev[b, m] = acc + accn, DMA'd [32, 256].

  M is built with a single bf16 pass (x, T pre-cast on host): the
  resulting quantization of M is shared by every downstream consumer,
  and keeps every matmul 16-bit so walrus ldw-opt (LDWEIGHTS dedupe)
  stays enabled.
"""

import sys

sys.path.insert(0, "/opt/trn_rl_repo")

import os
import numpy as np
import ml_dtypes

LDW_OPT = int(os.environ.get("MBD_LDW_OPT", "0"))
LDW_DEDUP = int(os.environ.get("MBD_LDW_DEDUP", "0"))
ACT_EVERY = int(os.environ.get("MBD_ACT_EVERY", "3"))
GP_ACC = int(os.environ.get("MBD_GP_ACC", "1"))
R_BUFS = int(os.environ.get("MBD_R_BUFS", "3"))
E_BUFS = int(os.environ.get("MBD_E_BUFS", "4"))
PSL1_BUFS = int(os.environ.get("MBD_PSL1_BUFS", "6"))
PSMT_BUFS = int(os.environ.get("MBD_PSMT_BUFS", "2"))
ALT_ORDER = int(os.environ.get("MBD_ALT_ORDER", "1"))
P_HEAD = int(os.environ.get("MBD_P_HEAD", "8"))

N = 256
IN_FEATURES = 1024
B_EXTRA = 256
K = 16
N_CORES = 8
B_LOCAL = B_EXTRA // N_CORES          # 32 features per core
BK = B_LOCAL * K                      # 512 = (b_local, k) flattened
N_CHUNKS = BK // 128                  # 4 partition chunks of (b,k)
I_CHUNKS = IN_FEATURES // 128         # 8 contraction chunks
P_MAX = 32
EW_MAX = 2048                         # max P*w elements per e buffer

_COMPILED = None


def _groups():
    gs = []
    n0 = 0
    while n0 < N:
        if n0 < 128:
            p = P_HEAD
        elif n0 < 192:
            p = 16
        else:
            p = 32
        gs.append((n0, p, N - n0))
        n0 += p
    return gs


def _apply_tile_drain_patch():
    """walrus in this container caps Drain (CTRL) instructions at one sem
    wait; Tile's end-of-kernel drain carries one wait per outstanding proc.
    Split the waits across a chain of drains."""
    from concourse import mybir, tile
    from concourse.vector_clock import ScopedClock

    def _drain_and_barrier(self, tick_clock, wait_clock):
        drain_inst = self.nc.sync.drain()
        wait_clock.add_sem_waits(
            drain_inst.ins, ScopedClock({None: tick_clock.global_clock})
        )
        si = drain_inst.ins.sync_info
        if si is not None and si.on_wait and len(si.on_wait) > 1:
            waits = list(si.on_wait)
            drain_inst.ins.sync_info = mybir.SyncInfo(
                on_wait=[waits[0]], on_update=list(si.on_update or [])
            )
            for w in waits[1:]:
                d = self.nc.sync.drain()
                d.ins.sync_info = mybir.SyncInfo(on_wait=[w], on_update=[])

        self.nc.all_engine_barrier()
        assert self.sems is not None
        popped = self.nc._tile_sem_poison_stack.pop()
        assert popped is self._sem_poison
        self.nc.clear_and_free_semaphores(list(self.sems.allocated().values()))
        self.nc.all_engine_barrier()

    tile.TileContext._drain_and_barrier = _drain_and_barrier


def _split_multi_waits(nc, max_waits=1):
    """This walrus build accepts at most one sync wait per instruction.
    Hoist extra waits onto NoOp instructions inserted just before the
    offending instruction in the same engine's stream."""
    from concourse import mybir

    cnt = 0
    for blk in nc.main_func.blocks:
        insts = blk.instructions
        if not any(
            inst.sync_info is not None
            and inst.sync_info.on_wait
            and len(inst.sync_info.on_wait) > max_waits
            for inst in insts
        ):
            continue
        new_list = []
        for inst in insts:
            si = inst.sync_info
            if si is not None and si.on_wait and len(si.on_wait) > max_waits:
                waits = list(si.on_wait)
                for w in waits[:-max_waits]:
                    nop = mybir.InstNoOp(name=f"wsplit-{cnt}", ins=[], outs=[])
                    cnt += 1
                    nop.engine = inst.engine
                    nop.sync_info = mybir.SyncInfo(on_wait=[w], on_update=[])
                    new_list.append(nop)
                inst.sync_info = mybir.SyncInfo(
                    on_wait=waits[-max_waits:],
                    on_update=list(si.on_update or []),
                )
            new_list.append(inst)
        insts[:] = new_list
    return cnt


def _dedupe_ldweights(nc):
    """Drop back-to-back InstLdweights on the PE stream that reload the
    same stationary (our correction matmuls all use the const i_sb /
    w_sb tiles, written once at start). Waits/updates of a dropped LDW
    are merged onto the following instruction."""
    from concourse import mybir

    dropped = 0
    for blk in nc.main_func.blocks:
        insts = blk.instructions
        new_list = []
        prev_sig = None
        pending = None  # dropped-LDW sync to merge into next PE inst
        for inst in insts:
            eng = getattr(inst, "engine", None)
            if eng != mybir.EngineType.PE:
                new_list.append(inst)
                continue
            if isinstance(inst, mybir.InstLdweights):
                ap = inst.ins[0]
                memref = getattr(ap, "memref", "")
                sig = (memref, getattr(ap, "offset", None), str(ap.ap),
                       str(getattr(ap, "dtype", "")))
                const_stat = memref.startswith(("i_sb", "w_sb"))
                if const_stat and sig == prev_sig:
                    si = inst.sync_info
                    if si is not None and (si.on_wait or si.on_update):
                        if pending is None:
                            pending = ([], [])
                        pending[0].extend(si.on_wait or [])
                        pending[1].extend(si.on_update or [])
                    dropped += 1
                    continue
                prev_sig = sig
            if pending is not None:
                si = inst.sync_info
                ow = list(si.on_wait or []) if si else []
                ou = list(si.on_update or []) if si else []
                inst.sync_info = mybir.SyncInfo(
                    on_wait=ow + pending[0], on_update=ou + pending[1])
                pending = None
            new_list.append(inst)
        assert pending is None
        insts[:] = new_list
    return dropped


def _apply_ldw_opt_patch():
    """Let walrus dedupe back-to-back identical LDWEIGHTS (the compile
    path hardcodes --enable-ldw-opt=false; our correction matmuls reuse
    the same stationary consecutively)."""
    from concourse import bass_utils

    if getattr(bass_utils, "_mbd_ldw_patched", False):
        return
    orig = bass_utils.run_command

    def patched(argv, **kw):
        argv = ["--enable-ldw-opt=true" if a == "--enable-ldw-opt=false"
                else a for a in argv]
        return orig(argv, **kw)

    bass_utils.run_command = patched
    bass_utils._mbd_ldw_patched = True


def _build():
    from concourse import bass, mybir, tile

    _apply_tile_drain_patch()
    if LDW_OPT:
        _apply_ldw_opt_patch()
    A = mybir.AluOpType
    F32 = mybir.dt.float32
    F16 = mybir.dt.float16
    BF16 = mybir.dt.bfloat16
    Exp = mybir.ActivationFunctionType.Exp

    nc = bass.Bass()
    xt_d = nc.declare_dram_parameter("xT", [IN_FEATURES, N], BF16,
                                     isOutput=False)
    t_d = nc.declare_dram_parameter("Tsh", [IN_FEATURES, BK], BF16,
                                    isOutput=False)
    w_d = nc.declare_dram_parameter("W", [128, N_CHUNKS * B_LOCAL], F16,
                                    isOutput=False)
    i_d = nc.declare_dram_parameter("Ipad", [128, B_LOCAL], F16,
                                    isOutput=False)
    pm_d = nc.declare_dram_parameter("PM", [128, P_MAX * P_MAX], F16,
                                     isOutput=False)
    out_d = nc.declare_dram_parameter("out", [B_LOCAL, N], F32, isOutput=True)

    with tile.TileContext(nc) as tc:
        with (
            tc.tile_pool(name="const", bufs=1) as const_pool,
            tc.tile_pool(name="mt", bufs=1) as mt_pool,
            tc.tile_pool(name="r", bufs=R_BUFS) as r_pool,
            tc.tile_pool(name="e", bufs=1) as e_pool,
            tc.tile_pool(name="psmt", bufs=PSMT_BUFS, space="PSUM") as psmt_pool,
            tc.tile_pool(name="psl1", bufs=PSL1_BUFS, space="PSUM") as psl1_pool,
            tc.tile_pool(name="psacc", bufs=1, space="PSUM") as psacc_pool,
        ):
            # ---- load inputs ----
            xt = const_pool.tile([128, I_CHUNKS, N], BF16, tag="xt")
            nc.sync.dma_start(
                xt[:], xt_d.rearrange("(c p) m -> p c m", p=128))
            tsh = const_pool.tile([128, I_CHUNKS, BK], BF16, tag="tsh")
            nc.sync.dma_start(
                tsh[:], t_d.rearrange("(c p) m -> p c m", p=128))
            w_sb = const_pool.tile([128, N_CHUNKS * B_LOCAL], F16, tag="w")
            nc.sync.dma_start(w_sb[:], w_d[:])
            i_sb = const_pool.tile([128, B_LOCAL], F16, tag="ipad")
            nc.sync.dma_start(i_sb[:], i_d[:])
            pm_sb = const_pool.tile([128, P_MAX, P_MAX], F16, tag="pm")
            nc.sync.dma_start(
                pm_sb[:], pm_d.rearrange("b (j m) -> b j m", j=P_MAX))

            # e buffers: [128, EW_MAX] fp16, rows 32-128 stay zero so the
            # column-accumulation matmul can contract 128 partitions.
            e_bufs = []
            for i in range(E_BUFS):
                eb = e_pool.tile([128, EW_MAX], F16, tag=f"e{i}")
                if not GP_ACC:
                    # rows 32-128 must be zero for the 128-contraction
                    # column-accumulation matmul
                    for p0 in range(B_LOCAL, 128, 32):
                        nc.gpsimd.memset(eb[p0:p0 + 32, :], 0.0)
                e_bufs.append(eb)
            # -SS tile, rows 32-128 zero
            st = mt_pool.tile([128, N], F16, tag="st")
            for p0 in range(B_LOCAL, 128, 32):
                nc.gpsimd.memset(st[p0:p0 + 32, :], 0.0)

            # ---- MT[(b,k), m]: fp16 M and 2M copies per chunk ----
            mt_h, mt2_h, mt_f = [], [], []
            for c in range(N_CHUNKS):
                ps = psmt_pool.tile([128, N], F32)
                for ic in range(I_CHUNKS):
                    nc.tensor.matmul(
                        ps[:],
                        tsh[:, ic, 128 * c:128 * (c + 1)],
                        xt[:, ic, :],
                        start=(ic == 0),
                        stop=(ic == I_CHUNKS - 1),
                    )
                mh = mt_pool.tile([128, N], F16, tag=f"mth{c}")
                nc.vector.tensor_copy(mh[:], ps[:])
                m2 = mt_pool.tile([128, N], F16, tag=f"mt2h{c}")
                nc.vector.tensor_scalar(m2[:], ps[:], 2.0, None, A.mult)
                mf = mt_pool.tile([128, N], F32, tag=f"mtf{c}")
                nc.vector.tensor_copy(mf[:], ps[:])
                mt_h.append(mh)
                mt2_h.append(m2)
                mt_f.append(mf)

            # ---- SS[b, m] = sum_k M[m, b, k] via the fp16 selector ----
            ss_ps = psmt_pool.tile([B_LOCAL, N], F32, tag="ps")
            for c in range(N_CHUNKS):
                nc.tensor.matmul(
                    ss_ps[:], w_sb[:, B_LOCAL * c:B_LOCAL * (c + 1)],
                    mt_h[c][:], start=(c == 0), stop=(c == N_CHUNKS - 1))
            nc.vector.tensor_scalar(
                st[0:B_LOCAL, :], ss_ps[:], -1.0, None, A.mult)

            # ---- accumulators ----
            accn = mt_pool.tile([B_LOCAL, N], F32, tag="accn")
            if GP_ACC:
                acc_gp = mt_pool.tile([B_LOCAL, N], F32, tag="accgp")
                nc.gpsimd.memset(acc_gp[:], 0.0)
                acc_ps = None
            else:
                acc_ps = psacc_pool.tile([B_LOCAL, N], F32)
                nc.vector.memset(acc_ps[:], 0.0)

            # ---- main loop ----
            Abs = mybir.ActivationFunctionType.Abs
            flip = False
            gi = 0
            for n0, P, w in _groups():
                # ACT-path groups compute r = |d| on the Scalar engine
                # (per-n bias) and need no SS corrections in PSUM; the
                # rest use batched DVE 2*max ops plus corrections. Only
                # head groups qualify: per-n ACT ops on narrow tail
                # groups are fixed-cost-dominated.
                use_act = (ACT_EVERY > 0 and n0 < 128
                           and gi % ACT_EVERY == 1)
                r = r_pool.tile([128, N_CHUNKS, P, w], F16, tag="r")
                if use_act:
                    for c in range(N_CHUNKS):
                        for j in range(P):
                            nc.scalar.activation(
                                r[:, c, j, :], mt_h[c][:, n0:N], Abs,
                                bias=mt_f[c][:, n0 + j:n0 + j + 1],
                                scale=-1.0)
                else:
                    for c in range(N_CHUNKS):
                        in0 = mt_h[c][:, n0:N].unsqueeze(1).broadcast_to(
                            (128, P, w))
                        in1 = mt2_h[c][:, n0:n0 + P].unsqueeze(2).broadcast_to(
                            (128, P, w))
                        nc.vector.scalar_tensor_tensor(
                            r[:, c, :, :], in0, 2.0, in1, A.mult, A.max)

                eb = e_bufs[gi % E_BUFS]
                gi += 1
                e = eb[:, 0:P * w].rearrange("p (j m) -> p j m", j=P)
                rb = max(1, 512 // w)
                j0 = 0
                while j0 < P:
                    j1 = min(P, j0 + rb)
                    rr = j1 - j0
                    ps = psl1_pool.tile([B_LOCAL, rr, w], F32)
                    pw = min(P, w)

                    # (stationary, moving, out) triples; start on first,
                    # stop on last. The first matmul must cover the full
                    # [rr, w] region (start=True resets only what it
                    # writes), so the partial-width poison never leads.
                    # i_sb-stationary matmuls adjacent so the LDWEIGHTS
                    # dedupe pass drops reloads; order alternates
                    # between banks to dedupe across seams too.
                    pm_mm = (i_sb[:], pm_sb[:, j0:j1, 0:pw],
                             ps[:, :, 0:pw])
                    sel_mms_l = [
                        (w_sb[:, B_LOCAL * c:B_LOCAL * (c + 1)],
                         r[:, c, j0:j1, :], ps[:])
                        for c in range(N_CHUNKS)
                    ]
                    do_flip = ALT_ORDER and flip
                    flip = not flip
                    if use_act:
                        mms = (sel_mms_l[::-1] if do_flip else sel_mms_l
                               ) + [pm_mm]
                    else:
                        corr_mms = [
                            (i_sb[:],
                             st[:, n0:N].unsqueeze(1).broadcast_to(
                                 (128, rr, w)), ps[:]),
                            (i_sb[:],
                             st[:, n0 + j0:n0 + j1].unsqueeze(2).broadcast_to(
                                 (128, rr, w)), ps[:]),
                            pm_mm,
                        ]
                        mms = corr_mms + sel_mms_l
                        if do_flip:
                            mms = mms[::-1]
                    for idx, (stat, mov, outap) in enumerate(mms):
                        nc.tensor.matmul(
                            outap, stat, mov,
                            start=(idx == 0), stop=(idx == len(mms) - 1),
                            skip_group_check=True)

                    nc.scalar.activation(
                        e[0:B_LOCAL, j0:j1, :], ps[:], Exp,
                        bias=0.0, scale=-1.0)
                    # column sums: acc[b, m] += sum_j e[b, j, m]
                    if GP_ACC:
                        for j in range(j0, j1):
                            nc.gpsimd.tensor_tensor(
                                acc_gp[:, n0:N], acc_gp[:, n0:N],
                                e[0:B_LOCAL, j, :], A.add)
                    else:
                        nc.tensor.matmul(
                            acc_ps[:, n0:N].unsqueeze(1).broadcast_to(
                                (B_LOCAL, rr, w)),
                            i_sb[:], e[:, j0:j1, :],
                            start=False, stop=False, skip_group_check=True)
                    j0 = j1

                # row sums: accn[b, n0+j] = sum_m e[b, j, m]
                nc.vector.tensor_reduce(
                    accn[:, n0:n0 + P], e[0:B_LOCAL, :, :],
                    mybir.AxisListType.X, A.add)

            # ---- combine and write out ----
            accf = mt_pool.tile([B_LOCAL, N], F32, tag="accf")
            nc.vector.tensor_tensor(
                accf[:], accn[:], acc_gp[:] if GP_ACC else acc_ps[:], A.add)
            nc.sync.dma_start(out_d[:], accf[:])

    if LDW_DEDUP:
        _dedupe_ldweights(nc)
    _split_multi_waits(nc)
    return nc


def _selector() -> np.ndarray:
    w = np.zeros((128, N_CHUNKS, B_LOCAL), dtype=np.float32)
    for c in range(N_CHUNKS):
        for p in range(128):
            w[p, c, (128 * c + p) // K] = 1.0
    return w.reshape(128, N_CHUNKS * B_LOCAL)


def _poison() -> np.ndarray:
    pm = np.zeros((128, P_MAX, P_MAX), dtype=np.float32)
    for j in range(P_MAX):
        pm[:B_LOCAL, j, :j + 1] = 1e4
    return pm.reshape(128, P_MAX * P_MAX)


def _in_maps(x: np.ndarray, T: np.ndarray) -> list:
    xt = np.ascontiguousarray(x.T).astype(ml_dtypes.bfloat16)  # (1024, 256)
    w = _selector().astype(np.float16)
    ipad = np.zeros((128, B_LOCAL), dtype=np.float16)
    ipad[:B_LOCAL] = np.eye(B_LOCAL, dtype=np.float16)
    pm = _poison().astype(np.float16)
    in_maps = []
    for c in range(N_CORES):
        tsh = np.ascontiguousarray(
            T[:, c * B_LOCAL:(c + 1) * B_LOCAL, :].reshape(IN_FEATURES, BK)
        ).astype(ml_dtypes.bfloat16)
        in_maps.append({"xT": xt, "Tsh": tsh, "W": w, "Ipad": ipad,
                        "PM": pm})
    return in_maps


def kernel(x: np.ndarray, T: np.ndarray) -> np.ndarray:
    global _COMPILED
    from concourse.bass_utils import run_bass_kernel_spmd

    x = np.ascontiguousarray(x, dtype=np.float32)
    T = np.ascontiguousarray(T, dtype=np.float32)

    if _COMPILED is None:
        _COMPILED = _build()
    nc = _COMPILED

    res = run_bass_kernel_spmd(nc, _in_maps(x, T), core_ids=list(range(N_CORES)))

    out = np.empty((N, IN_FEATURES + B_EXTRA), dtype=np.float32)
    out[:, :IN_FEATURES] = x
    for c in range(N_CORES):
        blk = res.results[c]["out"]                      # (32, 256) = (b, m)
        out[:, IN_FEATURES + c * B_LOCAL:IN_FEATURES + (c + 1) * B_LOCAL] = blk.T
    return out


# revision 27
# speedup vs baseline: 1.5219x; 1.5219x over previous
"""Trainium2 kernel for MinibatchDiscrimination.

reference:
    M = einsum('ni,ibk->nbk', x, T)            # (256, 256, 16)
    l1[n,m,b] = sum_k |M[n,b,k] - M[m,b,k]|
    out[m,b]  = sum_n exp(-l1[n,m,b]) - 1      # (256, 256)
    return concat([x, out], axis=1)            # (256, 1280)

Sharding: tensor-parallel over the B_extra=256 feature dim -> 32 features
per core, no collectives. Each core computes out[:, shard] as [32, 256]
(batch on partitions), host transposes and concatenates with x.

Per-core dataflow (upper triangle, batched groups):
  |a-b| = 2*max(a,b) - a - b, so
  l1[n,m,b] = 2*sum_k max(M[n,bk], M[m,bk]) - SS[b,n] - SS[b,m]
  with SS[b,m] = sum_k M[m,b,k] (selector matmul on M).

  Groups of P consecutive n's (P in {8,16,32}, growing as w = 256-n0
  shrinks). Per group:
    DVE: 4 scalar_tensor_tensor ops (one per 128-partition (b,k) chunk):
         r[:, c, j, m] = max(2*M[:, m], 2*M[:, n0+j]) via broadcast APs,
         fp16.
    PE (per PSUM bank of r_b = floor(512/w) rows): -SS[m] and -SS[n]
         corrections plus a poison matmul (+1e4 at m <= n: kills
         sub-diagonal and self terms after exp), then 4 selector
         matmuls contracting the (b,k) partitions of r. All correction/
         accumulation matmuls use 128-row zero-padded stationaries and
         moving tiles: 32-row contractions stream at half rate.
    ACT: one exp per bank: e = exp(-psum), fp16, poisoned entries -> 0.
    DVE: one segmented tensor_reduce per group: row sums
         accn[b, n0+j] = sum_m e[b, j, m].
    PE:  per bank, a column-accumulation matmul with a j-broadcast
         (0-stride) PSUM out AP: acc[b, m] += sum_j e[b, j, m].
  out_dev[b, m] = acc + accn, DMA'd [32, 256].

  M is built with a single bf16 pass (x, T pre-cast on host): the
  resulting quantization of M is shared by every downstream consumer,
  and keeps every matmul 16-bit so walrus ldw-opt (LDWEIGHTS dedupe)
  stays enabled.
"""

import sys

sys.path.insert(0, "/opt/trn_rl_repo")

import os
import numpy as np
import ml_dtypes

LDW_OPT = int(os.environ.get("MBD_LDW_OPT", "0"))
LDW_DEDUP = int(os.environ.get("MBD_LDW_DEDUP", "0"))
ACT_EVERY = int(os.environ.get("MBD_ACT_EVERY", "3"))
GP_ACC = int(os.environ.get("MBD_GP_ACC", "0"))
R_BUFS = int(os.environ.get("MBD_R_BUFS", "3"))
E_BUFS = int(os.environ.get("MBD_E_BUFS", "4"))
PSL1_BUFS = int(os.environ.get("MBD_PSL1_BUFS", "6"))
PSMT_BUFS = int(os.environ.get("MBD_PSMT_BUFS", "2"))
ALT_ORDER = int(os.environ.get("MBD_ALT_ORDER", "1"))
P_HEAD = int(os.environ.get("MBD_P_HEAD", "8"))

N = 256
IN_FEATURES = 1024
B_EXTRA = 256
K = 16
N_CORES = 8
B_LOCAL = B_EXTRA // N_CORES          # 32 features per core
BK = B_LOCAL * K                      # 512 = (b_local, k) flattened
N_CHUNKS = BK // 128                  # 4 partition chunks of (b,k)
I_CHUNKS = IN_FEATURES // 128         # 8 contraction chunks
P_MAX = 32
EW_MAX = 2048                         # max P*w elements per e buffer

_COMPILED = None


def _groups():
    gs = []
    n0 = 0
    while n0 < N:
        if n0 < 128:
            p = P_HEAD
        elif n0 < 192:
            p = 16
        else:
            p = 32
        gs.append((n0, p, N - n0))
        n0 += p
    return gs


def _apply_tile_drain_patch():
    """walrus in this container caps Drain (CTRL) instructions at one sem
    wait; Tile's end-of-kernel drain carries one wait per outstanding proc.
    Split the waits across a chain of drains."""
    from concourse import mybir, tile
    from concourse.vector_clock import ScopedClock

    def _drain_and_barrier(self, tick_clock, wait_clock):
        drain_inst = self.nc.sync.drain()
        wait_clock.add_sem_waits(
            drain_inst.ins, ScopedClock({None: tick_clock.global_clock})
        )
        si = drain_inst.ins.sync_info
        if si is not None and si.on_wait and len(si.on_wait) > 1:
            waits = list(si.on_wait)
            drain_inst.ins.sync_info = mybir.SyncInfo(
                on_wait=[waits[0]], on_update=list(si.on_update or [])
            )
            for w in waits[1:]:
                d = self.nc.sync.drain()
                d.ins.sync_info = mybir.SyncInfo(on_wait=[w], on_update=[])

        self.nc.all_engine_barrier()
        assert self.sems is not None
        popped = self.nc._tile_sem_poison_stack.pop()
        assert popped is self._sem_poison
        self.nc.clear_and_free_semaphores(list(self.sems.allocated().values()))
        self.nc.all_engine_barrier()

    tile.TileContext._drain_and_barrier = _drain_and_barrier


def _split_multi_waits(nc, max_waits=1):
    """This walrus build accepts at most one sync wait per instruction.
    Hoist extra waits onto NoOp instructions inserted just before the
    offending instruction in the same engine's stream."""
    from concourse import mybir

    cnt = 0
    for blk in nc.main_func.blocks:
        insts = blk.instructions
        if not any(
            inst.sync_info is not None
            and inst.sync_info.on_wait
            and len(inst.sync_info.on_wait) > max_waits
            for inst in insts
        ):
            continue
        new_list = []
        for inst in insts:
            si = inst.sync_info
            if si is not None and si.on_wait and len(si.on_wait) > max_waits:
                waits = list(si.on_wait)
                for w in waits[:-max_waits]:
                    nop = mybir.InstNoOp(name=f"wsplit-{cnt}", ins=[], outs=[])
                    cnt += 1
                    nop.engine = inst.engine
                    nop.sync_info = mybir.SyncInfo(on_wait=[w], on_update=[])
                    new_list.append(nop)
                inst.sync_info = mybir.SyncInfo(
                    on_wait=waits[-max_waits:],
                    on_update=list(si.on_update or []),
                )
            new_list.append(inst)
        insts[:] = new_list
    return cnt


def _dedupe_ldweights(nc):
    """Drop back-to-back InstLdweights on the PE stream that reload the
    same stationary (our correction matmuls all use the const i_sb /
    w_sb tiles, written once at start). Waits/updates of a dropped LDW
    are merged onto the following instruction."""
    from concourse import mybir

    dropped = 0
    for blk in nc.main_func.blocks:
        insts = blk.instructions
        new_list = []
        prev_sig = None
        pending = None  # dropped-LDW sync to merge into next PE inst
        for inst in insts:
            eng = getattr(inst, "engine", None)
            if eng != mybir.EngineType.PE:
                new_list.append(inst)
                continue
            if isinstance(inst, mybir.InstLdweights):
                ap = inst.ins[0]
                memref = getattr(ap, "memref", "")
                sig = (memref, getattr(ap, "offset", None), str(ap.ap),
                       str(getattr(ap, "dtype", "")))
                const_stat = memref.startswith(("i_sb", "w_sb"))
                if const_stat and sig == prev_sig:
                    si = inst.sync_info
                    if si is not None and (si.on_wait or si.on_update):
                        if pending is None:
                            pending = ([], [])
                        pending[0].extend(si.on_wait or [])
                        pending[1].extend(si.on_update or [])
                    dropped += 1
                    continue
                prev_sig = sig
            if pending is not None:
                si = inst.sync_info
                ow = list(si.on_wait or []) if si else []
                ou = list(si.on_update or []) if si else []
                inst.sync_info = mybir.SyncInfo(
                    on_wait=ow + pending[0], on_update=ou + pending[1])
                pending = None
            new_list.append(inst)
        assert pending is None
        insts[:] = new_list
    return dropped


def _apply_ldw_opt_patch():
    """Let walrus dedupe back-to-back identical LDWEIGHTS (the compile
    path hardcodes --enable-ldw-opt=false; our correction matmuls reuse
    the same stationary consecutively)."""
    from concourse import bass_utils

    if getattr(bass_utils, "_mbd_ldw_patched", False):
        return
    orig = bass_utils.run_command

    def patched(argv, **kw):
        argv = ["--enable-ldw-opt=true" if a == "--enable-ldw-opt=false"
                else a for a in argv]
        return orig(argv, **kw)

    bass_utils.run_command = patched
    bass_utils._mbd_ldw_patched = True


def _build():
    from concourse import bass, mybir, tile

    _apply_tile_drain_patch()
    if LDW_OPT:
        _apply_ldw_opt_patch()
    A = mybir.AluOpType
    F32 = mybir.dt.float32
    F16 = mybir.dt.float16
    BF16 = mybir.dt.bfloat16
    Exp = mybir.ActivationFunctionType.Exp

    nc = bass.Bass()
    xt_d = nc.declare_dram_parameter("xT", [IN_FEATURES, N], BF16,
                                     isOutput=False)
    t_d = nc.declare_dram_parameter("Tsh", [IN_FEATURES, BK], BF16,
                                    isOutput=False)
    w_d = nc.declare_dram_parameter("W", [128, N_CHUNKS * B_LOCAL], F16,
                                    isOutput=False)
    i_d = nc.declare_dram_parameter("Ipad", [128, B_LOCAL], F16,
                                    isOutput=False)
    pm_d = nc.declare_dram_parameter("PM", [128, P_MAX * P_MAX], F16,
                                     isOutput=False)
    out_d = nc.declare_dram_parameter("out", [B_LOCAL, N], F32, isOutput=True)

    with tile.TileContext(nc) as tc:
        with (
            tc.tile_pool(name="const", bufs=1) as const_pool,
            tc.tile_pool(name="mt", bufs=1) as mt_pool,
            tc.tile_pool(name="r", bufs=R_BUFS) as r_pool,
            tc.tile_pool(name="e", bufs=1) as e_pool,
            tc.tile_pool(name="psmt", bufs=PSMT_BUFS, space="PSUM") as psmt_pool,
            tc.tile_pool(name="psl1", bufs=PSL1_BUFS, space="PSUM") as psl1_pool,
            tc.tile_pool(name="psacc", bufs=1, space="PSUM") as psacc_pool,
        ):
            # ---- load inputs ----
            xt = const_pool.tile([128, I_CHUNKS, N], BF16, tag="xt")
            nc.sync.dma_start(
                xt[:], xt_d.rearrange("(c p) m -> p c m", p=128))
            tsh = const_pool.tile([128, I_CHUNKS, BK], BF16, tag="tsh")
            nc.sync.dma_start(
                tsh[:], t_d.rearrange("(c p) m -> p c m", p=128))
            w_sb = const_pool.tile([128, N_CHUNKS * B_LOCAL], F16, tag="w")
            nc.sync.dma_start(w_sb[:], w_d[:])
            i_sb = const_pool.tile([128, B_LOCAL], F16, tag="ipad")
            nc.sync.dma_start(i_sb[:], i_d[:])
            pm_sb = const_pool.tile([128, P_MAX, P_MAX], F16, tag="pm")
            nc.sync.dma_start(
                pm_sb[:], pm_d.rearrange("b (j m) -> b j m", j=P_MAX))

            # e buffers: [128, EW_MAX] fp16, rows 32-128 stay zero so the
            # column-accumulation matmul can contract 128 partitions.
            e_bufs = []
            for i in range(E_BUFS):
                eb = e_pool.tile([128, EW_MAX], F16, tag=f"e{i}")
                if not GP_ACC:
                    # rows 32-128 must be zero for the 128-contraction
                    # column-accumulation matmul
                    for p0 in range(B_LOCAL, 128, 32):
                        nc.gpsimd.memset(eb[p0:p0 + 32, :], 0.0)
                e_bufs.append(eb)
            # -SS tile, rows 32-128 zero
            st = mt_pool.tile([128, N], F16, tag="st")
            for p0 in range(B_LOCAL, 128, 32):
                nc.gpsimd.memset(st[p0:p0 + 32, :], 0.0)

            # ---- MT[(b,k), m]: fp16 M and 2M copies per chunk ----
            mt_h, mt2_h, mt_f = [], [], []
            for c in range(N_CHUNKS):
                ps = psmt_pool.tile([128, N], F32)
                for ic in range(I_CHUNKS):
                    nc.tensor.matmul(
                        ps[:],
                        tsh[:, ic, 128 * c:128 * (c + 1)],
                        xt[:, ic, :],
                        start=(ic == 0),
                        stop=(ic == I_CHUNKS - 1),
                    )
                mh = mt_pool.tile([128, N], F16, tag=f"mth{c}")
                nc.vector.tensor_copy(mh[:], ps[:])
                m2 = mt_pool.tile([128, N], F16, tag=f"mt2h{c}")
                nc.vector.tensor_scalar(m2[:], ps[:], 2.0, None, A.mult)
                mf = mt_pool.tile([128, N], F32, tag=f"mtf{c}")
                nc.vector.tensor_copy(mf[:], ps[:])
                mt_h.append(mh)
                mt2_h.append(m2)
                mt_f.append(mf)

            # ---- SS[b, m] = sum_k M[m, b, k] via the fp16 selector ----
            ss_ps = psmt_pool.tile([B_LOCAL, N], F32, tag="ps")
            for c in range(N_CHUNKS):
                nc.tensor.matmul(
                    ss_ps[:], w_sb[:, B_LOCAL * c:B_LOCAL * (c + 1)],
                    mt_h[c][:], start=(c == 0), stop=(c == N_CHUNKS - 1))
            nc.vector.tensor_scalar(
                st[0:B_LOCAL, :], ss_ps[:], -1.0, None, A.mult)

            # ---- accumulators ----
            accn = mt_pool.tile([B_LOCAL, N], F32, tag="accn")
            if GP_ACC:
                acc_gp = mt_pool.tile([B_LOCAL, N], F32, tag="accgp")
                nc.gpsimd.memset(acc_gp[:], 0.0)
                acc_ps = None
            else:
                acc_ps = psacc_pool.tile([B_LOCAL, N], F32)
                nc.vector.memset(acc_ps[:], 0.0)

            # ---- main loop ----
            Abs = mybir.ActivationFunctionType.Abs
            flip = False
            gi = 0
            for n0, P, w in _groups():
                # ACT-path groups compute r = |d| on the Scalar engine
                # (per-n bias) and need no SS corrections in PSUM; the
                # rest use batched DVE 2*max ops plus corrections. Only
                # head groups qualify: per-n ACT ops on narrow tail
                # groups are fixed-cost-dominated.
                use_act = (ACT_EVERY > 0 and n0 < 128
                           and gi % ACT_EVERY == 1)
                r = r_pool.tile([128, N_CHUNKS, P, w], F16, tag="r")
                if use_act:
                    for c in range(N_CHUNKS):
                        for j in range(P):
                            nc.scalar.activation(
                                r[:, c, j, :], mt_h[c][:, n0:N], Abs,
                                bias=mt_f[c][:, n0 + j:n0 + j + 1],
                                scale=-1.0)
                else:
                    for c in range(N_CHUNKS):
                        in0 = mt_h[c][:, n0:N].unsqueeze(1).broadcast_to(
                            (128, P, w))
                        in1 = mt2_h[c][:, n0:n0 + P].unsqueeze(2).broadcast_to(
                            (128, P, w))
                        nc.vector.scalar_tensor_tensor(
                            r[:, c, :, :], in0, 2.0, in1, A.mult, A.max)

                eb = e_bufs[gi % E_BUFS]
                gi += 1
                e = eb[:, 0:P * w].rearrange("p (j m) -> p j m", j=P)
                rb = max(1, 512 // w)
                j0 = 0
                while j0 < P:
                    j1 = min(P, j0 + rb)
                    rr = j1 - j0
                    ps = psl1_pool.tile([B_LOCAL, rr, w], F32)
                    pw = min(P, w)

                    # (stationary, moving, out) triples; start on first,
                    # stop on last. The first matmul must cover the full
                    # [rr, w] region (start=True resets only what it
                    # writes), so the partial-width poison never leads.
                    # i_sb-stationary matmuls adjacent so the LDWEIGHTS
                    # dedupe pass drops reloads; order alternates
                    # between banks to dedupe across seams too.
                    pm_mm = (i_sb[:], pm_sb[:, j0:j1, 0:pw],
                             ps[:, :, 0:pw])
                    sel_mms_l = [
                        (w_sb[:, B_LOCAL * c:B_LOCAL * (c + 1)],
                         r[:, c, j0:j1, :], ps[:])
                        for c in range(N_CHUNKS)
                    ]
                    do_flip = ALT_ORDER and flip
                    flip = not flip
                    if use_act:
                        mms = (sel_mms_l[::-1] if do_flip else sel_mms_l
                               ) + [pm_mm]
                    else:
                        corr_mms = [
                            (i_sb[:],
                             st[:, n0:N].unsqueeze(1).broadcast_to(
                                 (128, rr, w)), ps[:]),
                            (i_sb[:],
                             st[:, n0 + j0:n0 + j1].unsqueeze(2).broadcast_to(
                                 (128, rr, w)), ps[:]),
                            pm_mm,
                        ]
                        mms = corr_mms + sel_mms_l
                        if do_flip:
                            mms = mms[::-1]
                    for idx, (stat, mov, outap) in enumerate(mms):
                        nc.tensor.matmul(
                            outap, stat, mov,
                            start=(idx == 0), stop=(idx == len(mms) - 1),
                            skip_group_check=True)

                    nc.scalar.activation(
                        e[0:B_LOCAL, j0:j1, :], ps[:], Exp,
                        bias=0.0, scale=-1.0)
                    # column sums: acc[b, m] += sum_j e[b, j, m]
                    if GP_ACC:
                        for j in range(j0, j1):
                            nc.gpsimd.tensor_tensor(
                                acc_gp[:, n0:N], acc_gp[:, n0:N],
                                e[0:B_LOCAL, j, :], A.add)
                    else:
                        nc.tensor.matmul(
                            acc_ps[:, n0:N].unsqueeze(1).broadcast_to(
                                (B_LOCAL, rr, w)),
                            i_sb[:], e[:, j0:j1, :],
                            start=False, stop=False, skip_group_check=True)
                    j0 = j1

                # row sums: accn[b, n0+j] = sum_m e[b, j, m]
                nc.vector.tensor_reduce(
                    accn[:, n0:n0 + P], e[0:B_LOCAL, :, :],
                    mybir.AxisListType.X, A.add)

            # ---- combine and write out ----
            accf = mt_pool.tile([B_LOCAL, N], F32, tag="accf")
            nc.vector.tensor_tensor(
                accf[:], accn[:], acc_gp[:] if GP_ACC else acc_ps[:], A.add)
            nc.sync.dma_start(out_d[:], accf[:])

    if LDW_DEDUP:
        _dedupe_ldweights(nc)
    _split_multi_waits(nc)
    return nc


def _selector() -> np.ndarray:
    w = np.zeros((128, N_CHUNKS, B_LOCAL), dtype=np.float32)
    for c in range(N_CHUNKS):
        for p in range(128):
            w[p, c, (128 * c + p) // K] = 1.0
    return w.reshape(128, N_CHUNKS * B_LOCAL)


def _poison() -> np.ndarray:
    pm = np.zeros((128, P_MAX, P_MAX), dtype=np.float32)
    for j in range(P_MAX):
        pm[:B_LOCAL, j, :j + 1] = 1e4
    return pm.reshape(128, P_MAX * P_MAX)


def _in_maps(x: np.ndarray, T: np.ndarray) -> list:
    xt = np.ascontiguousarray(x.T).astype(ml_dtypes.bfloat16)  # (1024, 256)
    w = _selector().astype(np.float16)
    ipad = np.zeros((128, B_LOCAL), dtype=np.float16)
    ipad[:B_LOCAL] = np.eye(B_LOCAL, dtype=np.float16)
    pm = _poison().astype(np.float16)
    in_maps = []
    for c in range(N_CORES):
        tsh = np.ascontiguousarray(
            T[:, c * B_LOCAL:(c + 1) * B_LOCAL, :].reshape(IN_FEATURES, BK)
        ).astype(ml_dtypes.bfloat16)
        in_maps.append({"xT": xt, "Tsh": tsh, "W": w, "Ipad": ipad,
                        "PM": pm})
    return in_maps


def kernel(x: np.ndarray, T: np.ndarray) -> np.ndarray:
    global _COMPILED
    from concourse.bass_utils import run_bass_kernel_spmd

    x = np.ascontiguousarray(x, dtype=np.float32)
    T = np.ascontiguousarray(T, dtype=np.float32)

    if _COMPILED is None:
        _COMPILED = _build()
    nc = _COMPILED

    res = run_bass_kernel_spmd(nc, _in_maps(x, T), core_ids=list(range(N_CORES)))

    out = np.empty((N, IN_FEATURES + B_EXTRA), dtype=np.float32)
    out[:, :IN_FEATURES] = x
    for c in range(N_CORES):
        blk = res.results[c]["out"]                      # (32, 256) = (b, m)
        out[:, IN_FEATURES + c * B_LOCAL:IN_FEATURES + (c + 1) * B_LOCAL] = blk.T
    return out
